# revision 27
# baseline (speedup 1.0000x reference)
"""Trainium2 Bass kernel for nn_AttentionLayer (sparse_attention).

Computes, for inputs lstm_lt (B,L,H), lstm_rt (B,R,H), atten_W (H,D),
diagnoal_W (1,1,D):

    atten_lt = tanh(lstm_lt @ W) * diag
    atten_rt = tanh(lstm_rt @ W)
    out      = softmax(atten_lt @ atten_rt^T, axis=-1)      # (B, L, R)

Strategy: pure data parallel over the batch dim across 8 NeuronCores
(8 batches per core).  The host pre-transposes the lstm tensors to
(B, H, L), pre-scales by 2.9 (so N(0,1) values clear e3m4's subnormal
zone; min normal 0.25) and ships BOTH as float8_e3m4 — the 1/2.9 is
folded into the tanh's ACT scale for free.  Per batch, the projections
are computed in transposed layout P^T = W^T @ lstm^T (D on partitions),
exactly the layout the scores matmul needs for both operands.

Measured facts driving the design (see microbench.py):
- PE floor 98,304 cyc/iter = 41.5us; irreducible: fp8 DoubleRow needs
  e4m3 operands and any e4m3 tensor blows the 2e-2 error gate.
- The DVE queue is the real full-kernel bottleneck (~347ns issue per
  instruction, partially serialized with engine time and cross-engine
  waits), so the softmax row sums ride the exp's ACT accum_out and the
  DVE carries only diag-mult + reciprocal + normalize-mults.
- H and L are consumed in permuted orders ("(p k)" / "(p i)") so every
  DMA touches contiguous per-partition runs (4x fewer descriptors).
- GPSIMD is unusable for per-batch work (~3-5us launch per Q7 op).
Softmax skips the max-subtraction (scores are O(1), exp cannot
overflow in fp32).
"""

import numpy as np
import ml_dtypes

B, L, R, H, D = 64, 512, 512, 512, 256
N_CORES = 8
KB = B // N_CORES  # batches per core

_CACHE = {}

OUT_BF16 = True  # device writes 16-bit probs; host casts to fp32
RT_E3M4 = True  # ship lstm_rt as float8_e3m4
LT_E3M4 = True  # ship lstm_lt as float8_e3m4 too
# Pre-scale inputs before the e3m4 cast so N(0,1) values clear the
# subnormal zone (e3m4 min normal = 0.25); max|x| ~5.42 so 2.9 stays
# inside e3m4 max 15.5.  The 1/2.9 is folded into the tanh's ACT scale.
PRESCALE = 2.9
E3_MAX = 15.5


def _build_program(reps=1, hw_loop_n=None):
    import concourse.bass as bass  # noqa: F401
    import concourse.tile as tile
    from concourse import bacc, mybir

    f32 = mybir.dt.float32
    bf16 = mybir.dt.float16  # fp16: same speed as bf16, 8x finer mantissa
    rt_dt = mybir.dt.float8e3 if RT_E3M4 else bf16
    lt_dt = mybir.dt.float8e3 if LT_E3M4 else bf16
    out_dt = bf16 if OUT_BF16 else f32
    unscale = 1.0 / PRESCALE
    AF = mybir.ActivationFunctionType
    Alu = mybir.AluOpType

    nc = bacc.Bacc(
        "TRN2",
        target_bir_lowering=False,
        debug=False,
        enable_asserts=False,
        num_devices=N_CORES,
    )
    ltT = nc.dram_tensor("ltT", [KB, H, L], lt_dt, kind="ExternalInput").ap()
    rtT = nc.dram_tensor("rtT", [KB, H, R], rt_dt, kind="ExternalInput").ap()
    w = nc.dram_tensor("w", [H, D], bf16, kind="ExternalInput").ap()
    diag = nc.dram_tensor("diag", [D, 1], f32, kind="ExternalInput").ap()
    out = nc.dram_tensor("out", [KB, L, R], out_dt, kind="ExternalOutput").ap()

    HT = H // 128  # 4 contraction tiles
    DT = D // 128  # 2 projection-output tiles
    LT = L // 128  # 4 score-output tiles

    with tile.TileContext(nc) as tc:
        with (
            tc.tile_pool(name="const", bufs=1) as cpool,
            tc.tile_pool(name="ins", bufs=4) as inpool,
            tc.tile_pool(name="proj", bufs=4) as ppool,
            tc.tile_pool(name="soft", bufs=4) as spool,
            tc.tile_pool(name="stats", bufs=4) as stpool,
            tc.tile_pool(name="outs", bufs=4) as opool,
            tc.tile_pool(name="ppsum", bufs=2, space="PSUM") as ppsum,
            tc.tile_pool(name="spsum", bufs=2, space="PSUM") as spsum,
        ):
            # H is consumed in a permuted order (partition p, slot k <-> row
            # 4p+k) so each partition's slice of every H-major DMA is one
            # contiguous run; contraction over H is order-invariant as long
            # as w and lt/rt agree, which they do below.
            w_sb = cpool.tile([128, HT, D], bf16)
            nc.sync.dma_start(w_sb[:], w.rearrange("(p k) d -> p k d", k=HT))
            diag_sb = cpool.tile([128, DT], f32)
            nc.sync.dma_start(diag_sb[:], diag.rearrange("(t p) o -> p (t o)", p=128))

            # Warm-up while the first loads are in flight: dummy matmuls push
            # the PE HAM past its ~3.4us activity window so real matmuls start
            # at 2.4 GHz, and a dummy tanh pulls the ACT table load (~2.7us)
            # off batch 0's critical path.  Inputs are junk SBUF; the PSUM
            # scratch slot is released before the first real projection needs
            # it (and every real accumulation starts with start=True anyway).
            junk = cpool.tile([128, 512], bf16)
            nc.gpsimd.memset(junk[:], 0.0)
            warm_ps = ppsum.tile([128, DT, L], f32, name="warm_ps", tag="ps")
            for _ in range(10):
                nc.tensor.matmul(
                    warm_ps[:, 0, :], junk[:, 0:128], junk[:], start=True, stop=True
                )
            warm_act = cpool.tile([128, 1], bf16)
            nc.scalar.activation(warm_act[:], junk[:, 0:1], AF.Tanh)

            first = [True]

            def emit_load_proj(b):
                """DMA loads + projection matmuls + tanh + diag for batch b.
                Returns (pld, prt) bf16 tiles [(128, DT, L/R)].

                GPSIMD is deliberately NOT used here: each Q7 software op
                costs ~4-5us of launch overhead on real hw (measured: 24
                Pool ops/rep pushed the whole kernel from 57us to 136us)."""
                lt_sb = inpool.tile([128, HT, L], lt_dt, name="lt_sb")
                rt_sb = inpool.tile([128, HT, R], rt_dt, name="rt_sb")
                nc.scalar.dma_start(
                    lt_sb[:], ltT[b].rearrange("(p k) l -> p k l", k=HT)
                )
                nc.sync.dma_start(
                    rt_sb[:], rtT[b].rearrange("(p k) l -> p k l", k=HT)
                )

                ps_l = ppsum.tile([128, DT, L], f32, name="ps", tag="ps")
                for dd in range(DT):
                    dsl = slice(dd * 128, (dd + 1) * 128)
                    for k in range(HT):
                        nc.tensor.matmul(
                            ps_l[:, dd, :], w_sb[:, k, dsl], lt_sb[:, k, :],
                            start=(k == 0), stop=(k == HT - 1),
                        )
                plt = ppool.tile([128, DT, L], bf16, name="plt")
                nc.scalar.activation(plt[:], ps_l[:], AF.Tanh, scale=unscale)
                pld = ppool.tile([128, DT, L], bf16, name="pld")
                for dd in range(DT):
                    nc.vector.tensor_scalar_mul(
                        pld[:, dd, :], plt[:, dd, :], diag_sb[:, dd : dd + 1]
                    )

                ps_r = ppsum.tile([128, DT, R], f32, name="ps_r", tag="ps")
                for dd in range(DT):
                    dsl = slice(dd * 128, (dd + 1) * 128)
                    for k in range(HT):
                        nc.tensor.matmul(
                            ps_r[:, dd, :], w_sb[:, k, dsl], rt_sb[:, k, :],
                            start=(k == 0), stop=(k == HT - 1),
                        )
                prt = ppool.tile([128, DT, R], bf16, name="prt")
                nc.scalar.activation(prt[:], ps_r[:], AF.Tanh, scale=unscale)
                return pld, prt

            def emit_scores_softmax(b, pld, prt, last=False):
                """Scores + softmax + store for batch b, two L-halves of
                2 PSUM banks each.  Row sums ride on the exp's ACT
                accumulator (4-way split exp), so the DVE queue — the
                measured full-kernel bottleneck at ~347ns issue per
                instruction — carries only one reciprocal and four
                normalize-mults per batch, none of it PE-critical."""
                e = spool.tile([128, LT, R], bf16, name="e")
                ssum = stpool.tile([128, LT], f32, name="ssum")
                o = opool.tile([128, LT, R], out_dt, name="o")
                # L is tiled in a permuted order too: score row 4p+i lands on
                # psum partition p of slot i (stationary = stride-LT slice of
                # pld), making each partition's 4 output rows contiguous in
                # HBM -> 4KB store descriptors.  Softmax is row-local, so
                # only the stationary slice and the store AP change.
                for h in range(LT // 2):
                    ss = spsum.tile([128, 2, R], f32, name="ss", tag="ss")
                    for ii in range(2):
                        i = 2 * h + ii
                        for dd in range(DT):
                            nc.tensor.matmul(
                                ss[:, ii, :],
                                pld[:, dd, :].rearrange(
                                    "p (q i) -> p i q", i=LT
                                )[:, i, :],
                                prt[:, dd, :],
                                start=(dd == 0), stop=(dd == DT - 1),
                            )
                    for ii in range(2):
                        i = 2 * h + ii
                        nc.scalar.activation(
                            e[:, i, :], ss[:, ii, :], AF.Exp,
                            accum_out=ssum[:, i : i + 1],
                        )
                rcp = stpool.tile([128, LT], f32, name="rcp")
                nc.vector.reciprocal(rcp[:], ssum[:])
                for i in range(LT):
                    nc.vector.tensor_scalar_mul(
                        o[:, i, :], e[:, i, :], rcp[:, i : i + 1]
                    )
                nc.sync.dma_start(
                    out[b].rearrange("(p i) r -> p i r", i=LT), o[:]
                )

            # Two-stage software pipeline: proj(b+1) is emitted before
            # scores(b) so the PE stream never waits on tanh.  (A 3-stage
            # variant measured the same within noise; the kernel is
            # power-throttle-bound, not latency-bound.)
            PIPE = 1

            def emit_reps(n, last):
                batches = [bb for _ in range(n) for bb in range(KB)]
                pend = []
                for b in batches:
                    pend.append((b, *emit_load_proj(b)))
                    if len(pend) > PIPE:
                        emit_scores_softmax(*pend.pop(0))
                while pend:
                    emit_scores_softmax(*pend.pop(0), last=last and not pend)

            if hw_loop_n is None:
                emit_reps(reps, last=True)
            else:
                with tc.For_i(0, hw_loop_n):
                    emit_reps(reps, last=True)

    nc.compile()
    return nc


def _get_program(reps=1, hw_loop_n=None):
    key = ("nc", reps, hw_loop_n, RT_E3M4)
    if key not in _CACHE:
        _CACHE[key] = _build_program(reps, hw_loop_n)
    return _CACHE[key]


def _get_runner(reps=1):
    """Build (once) a jitted shard_map executable over the 8 cores.

    Returns run(in_maps) -> list[dict] of per-core outputs.
    """
    key = ("runner", reps)
    if key in _CACHE:
        return _CACHE[key]

    import jax
    from jax.sharding import Mesh, PartitionSpec
    from jax.experimental.shard_map import shard_map
    import concourse.mybir as mybir
    from concourse.bass2jax import _bass_exec_p, install_neuronx_cc_hook

    nc = _get_program(reps)
    install_neuronx_cc_hook()

    partition_name = nc.partition_id_tensor.name if nc.partition_id_tensor else None
    in_names, out_names, out_avals, zero_outs = [], [], [], []
    for alloc in nc.m.functions[0].allocations:
        if not isinstance(alloc, mybir.MemoryLocationSet):
            continue
        name = alloc.memorylocations[0].name
        if alloc.kind == "ExternalInput":
            if name != partition_name:
                in_names.append(name)
        elif alloc.kind == "ExternalOutput":
            shape = tuple(alloc.tensor_shape)
            dtype = mybir.dt.np(alloc.dtype)
            out_names.append(name)
            out_avals.append(jax.core.ShapedArray(shape, dtype))
            zero_outs.append(np.zeros(shape, dtype))
    n_params = len(in_names)
    all_in_names = list(in_names) + list(out_names)
    if partition_name is not None:
        all_in_names.append(partition_name)

    def _body(*args):
        operands = list(args)
        if partition_name is not None:
            from concourse.bass2jax import partition_id_tensor

            operands.append(partition_id_tensor())
        return tuple(
            _bass_exec_p.bind(
                *operands,
                out_avals=tuple(out_avals),
                in_names=tuple(all_in_names),
                out_names=tuple(out_names),
                lowering_input_output_aliases=(),
                sim_require_finite=True,
                sim_require_nnan=True,
                nc=nc,
            )
        )

    devices = jax.devices()[:N_CORES]
    mesh = Mesh(np.asarray(devices), ("core",))
    in_specs = (PartitionSpec("core"),) * (n_params + len(out_names))
    out_specs = (PartitionSpec("core"),) * len(out_names)
    sharded = jax.jit(
        shard_map(
            _body, mesh=mesh, in_specs=in_specs, out_specs=out_specs, check_rep=False
        ),
        keep_unused=True,
    )
    concat_zeros = [
        np.zeros((N_CORES * z.shape[0], *z.shape[1:]), z.dtype) for z in zero_outs
    ]

    def run(in_maps):
        concat_in = [
            np.concatenate([np.asarray(in_maps[c][nm]) for c in range(N_CORES)], axis=0)
            for nm in in_names
        ]
        outs = sharded(*concat_in, *concat_zeros)
        return [
            {
                nm: np.asarray(outs[i]).reshape(N_CORES, *out_avals[i].shape)[c]
                for i, nm in enumerate(out_names)
            }
            for c in range(N_CORES)
        ]

    _CACHE[key] = run
    return run


def _prescale_cast(x, dt):
    y = np.clip(np.asarray(x, np.float32) * PRESCALE, -E3_MAX, E3_MAX)
    return y.astype(dt)


def _make_in_maps(inputs_np):
    bf = np.float16
    rt_np = ml_dtypes.float8_e3m4 if RT_E3M4 else bf
    lt_np = ml_dtypes.float8_e3m4 if LT_E3M4 else bf
    ltT = np.ascontiguousarray(
        _prescale_cast(inputs_np["lstm_lt"], lt_np).transpose(0, 2, 1)
    )
    rtT = np.ascontiguousarray(
        _prescale_cast(inputs_np["lstm_rt"], rt_np).transpose(0, 2, 1)
    )
    w = np.ascontiguousarray(np.asarray(inputs_np["atten_W"]).astype(bf))
    diag = np.ascontiguousarray(np.asarray(inputs_np["diagnoal_W"]).astype(np.float32).reshape(D, 1))
    return [
        {"ltT": ltT[c * KB : (c + 1) * KB], "rtT": rtT[c * KB : (c + 1) * KB], "w": w, "diag": diag}
        for c in range(N_CORES)
    ]


def _run(lstm_lt, lstm_rt, atten_W, diagnoal_W, reps=1):
    in_maps = _make_in_maps(
        {
            "lstm_lt": lstm_lt,
            "lstm_rt": lstm_rt,
            "atten_W": atten_W,
            "diagnoal_W": diagnoal_W,
        }
    )
    res = _get_runner(reps)(in_maps)
    out = np.concatenate([res[c]["out"] for c in range(N_CORES)], axis=0)
    return out.astype(np.float32), None


def kernel(lstm_lt, lstm_rt, atten_W, diagnoal_W):
    out, _ = _run(lstm_lt, lstm_rt, atten_W, diagnoal_W)
    return out

